# revision 2
# baseline (speedup 1.0000x reference)
"""Trainium2 Bass kernel for the ChessTransformer problem — v2.

Strategy: pure data-parallel over batch (B=2048 -> 256 samples/core on 8
NeuronCores). Per core, samples are processed in SBUF-resident chunks of 28
(4 G-tiles of 7 samples) that flow through embed + all 12 layers + head
feature gather without touching DRAM; only per-layer weights are streamed
(re-loaded once per chunk, ~34 MB/chunk, hidden under compute).

Activations are feature-major: 8 d-tiles of [128, chunk_cols] with residual
kept in f32 (xf) and a bf16 mirror (xb) feeding the PE.

Attention per head avoids separate q/k projections: host precomputes
A_h = qkv0_h @ qkv1_h^T / sqrt(dh) so scores = x^T A x needs one projection
(z = A^T x) plus per-sample matmuls that share their stationary operand
(x_h,g) between the score matmul and the v matmul. Softmax is over the
partition axis via a ones-matmul column sum; normalization uses the fast
DVE reciprocal and a bf16 multiply.

All ACT usage stays inside the natural_log_exp_and_others table set
(Exp, Ln, Prelu/parametric_relu, Copy, Square): LayerNorm rstd is
exp(-0.5*ln(var+eps)) and the final sigmoid is 1/(1+exp(-x)) via Exp +
DVE reciprocal, so no activation-table reloads occur anywhere.
"""

import sys

sys.path.insert(0, "/opt/trn_rl_repo")

import numpy as np
import ml_dtypes

import concourse.bacc as bacc
import concourse.bass as bass
import concourse.mybir as mybir
from concourse import tile
from concourse.bass_utils import run_bass_kernel_spmd

F32 = mybir.dt.float32
BF16 = mybir.dt.bfloat16
AF = mybir.ActivationFunctionType
ALU = mybir.AluOpType

D = 1024
H = 8
DH = 128
T = 71
KV = 81  # 17 fen rows + 64 pos rows
G = 7  # samples per compute tile (G*T = 497 <= 512 PSUM cols)
N_CORES = 8
EPS = 1e-5

TIMING_REPEAT = 1   # >1: re-run the chunk loop (timing experiments only)
CHUNK = 28          # samples per resident chunk (= 4 G-tiles)
CN = CHUNK * T      # 1988 cols
NCHUNK = 9          # full chunks per core (9*28 = 252)
TAIL = 4            # leftover samples (1 tile of G=4)
BC = NCHUNK * CHUNK + TAIL  # 256


def _bf(a):
    return np.ascontiguousarray(a.astype(ml_dtypes.bfloat16))


def _f32(a):
    return np.ascontiguousarray(a.astype(np.float32))


def host_prep(inputs, n_cores=N_CORES):
    """Build per-core input maps + flags from full-size inputs."""
    fen = np.asarray(inputs["fen"]).astype(np.int64)
    move = np.asarray(inputs["move"]).astype(np.int64)
    B = fen.shape[0]
    Bc = B // n_cores
    L = np.asarray(inputs["qkv"]).shape[0]

    rank_emb = np.asarray(inputs["rank_emb"], np.float32)
    file_emb = np.asarray(inputs["file_emb"], np.float32)
    fen_emb = np.asarray(inputs["fen_emb"], np.float32)
    move_emb = np.asarray(inputs["move_emb"], np.float32)
    abs_emb = np.asarray(inputs["abs_emb"], np.float32)
    qkv = np.asarray(inputs["qkv"], np.float32)
    ff1 = np.asarray(inputs["ff1"], np.float32)
    ff2 = np.asarray(inputs["ff2"], np.float32)
    W1 = np.asarray(inputs["W1"], np.float32)
    b1 = np.asarray(inputs["b1"], np.float32)
    W2 = np.asarray(inputs["W2"], np.float32)
    b2 = np.asarray(inputs["b2"], np.float32)
    lng = np.asarray(inputs["ln_emb_g"], np.float32)
    lnb = np.asarray(inputs["ln_emb_b"], np.float32)
    log = np.asarray(inputs["ln_out_g"], np.float32)
    lob = np.asarray(inputs["ln_out_b"], np.float32)

    pos = (rank_emb + file_emb).reshape(64, D)

    # table + per-token-position constants
    vtab = np.concatenate([fen_emb, 0.58 * pos], axis=0)  # (81, D)
    C = np.empty((T, D), np.float32)
    C[:64] = 0.5 * pos + abs_emb[:64]
    C[64:69] = abs_emb[64:69]
    C[69:71] = 0.58 * move_emb + abs_emb[69:71]

    # count matrix (two-hot embedding weights), cols = b*71 + t
    cnt = np.zeros((KV, B, T), np.float32)
    bidx = np.arange(B)[:, None]
    np.add.at(cnt, (fen[:, :64], bidx, np.arange(64)[None, :]), 0.5)
    np.add.at(cnt, (fen[:, 64:128], bidx, np.arange(64)[None, :]), 0.5)
    np.add.at(cnt, (fen[:, 128:133], bidx, np.arange(64, 69)[None, :]), 1.0)
    np.add.at(cnt, (17 + move, bidx, np.arange(69, 71)[None, :]), 1.0)
    cnt = cnt.reshape(KV, B * T)

    # const replicated G times: feature-major d-tiles [8, 128, G*71]
    Cfm = C.T.reshape(8, 128, T)
    cstr = np.tile(Cfm, (1, 1, G))

    scale = np.sqrt(np.float32(DH))
    # fused score weight: z = wz.T @ x gives scores = x^T z = q.k/scale
    # wz[l,:,h] = qkv0 @ qkv1^T / scale  (as lhsT: out = lhsT.T @ rhs)
    wz = np.einsum("lhic,lhjc->lihj", qkv[:, 0], qkv[:, 1]).reshape(L, 128, H * 128)
    wz = wz / scale
    # v weight (moving operand): vt[s,d'] = sum_c x[c,s] qkv2[c,d']
    wv = qkv[:, 2].transpose(0, 2, 1, 3).reshape(L, 128, H * 128)
    wf1 = (
        ff1.reshape(L, H, 8, 128, DH).transpose(0, 3, 1, 2, 4).reshape(L, 128, H * 8 * 128)
    )
    wf2 = ff2.transpose(0, 2, 1, 3).reshape(L, 128, H * 128)

    w1t = W1.T.reshape(16, 128, 2 * D)  # [k, p, out]
    w2s = W2.reshape(16, 128).T  # [128, 16]
    hb1 = b1.reshape(16, 128).T  # [128, 16]
    hg = log.reshape(16, 128).T
    hbt = lob.reshape(16, 128).T
    gemb = lng.reshape(8, 128).T  # [128, 8]
    bemb = lnb.reshape(8, 128).T

    flags = dict(
        apply_gemb=not (np.all(lng == 1.0) and np.all(lnb == 0.0)),
        apply_ghead=not (np.all(log == 1.0) and np.all(lob == 0.0)),
        use_b1=bool(np.any(b1 != 0.0)),
        use_b2=bool(np.any(b2 != 0.0)),
        Bc=Bc,
        L=L,
    )

    shared = {
        "ident": _bf(np.eye(128, dtype=np.float32)),
        "vtab": _bf(vtab),
        "cstr": _f32(cstr),
        "wz": _bf(wz),
        "wv": _bf(wv),
        "wf1": _bf(wf1),
        "wf2": _bf(wf2),
        "w1t": _bf(w1t),
        "w2s": _bf(w2s),
        "hb1": _f32(hb1),
        "hg": _f32(hg),
        "hbt": _f32(hbt),
        "gemb": _f32(gemb),
        "bemb": _f32(bemb),
        "b2": _f32(b2.reshape(1, 1)),
    }
    cnt_bf = _bf(cnt)
    in_maps = []
    for c in range(n_cores):
        m = dict(shared)
        m["cnt"] = np.ascontiguousarray(cnt_bf[:, c * Bc * T : (c + 1) * Bc * T])
        in_maps.append(m)
    return in_maps, flags


def build_program(flags):
    """Emit the full per-core program."""
    Bc = flags["Bc"]
    L = flags["L"]
    TOK = Bc * T
    assert Bc == BC

    nc = bacc.Bacc("TRN2", target_bir_lowering=False, debug=False)

    cnt_d = nc.dram_tensor("cnt", [KV, TOK], BF16, kind="ExternalInput")
    ident_d = nc.dram_tensor("ident", [128, 128], BF16, kind="ExternalInput")
    vtab_d = nc.dram_tensor("vtab", [KV, D], BF16, kind="ExternalInput")
    cstr_d = nc.dram_tensor("cstr", [8, 128, G * T], F32, kind="ExternalInput")
    wz_d = nc.dram_tensor("wz", [L, 128, H * 128], BF16, kind="ExternalInput")
    wv_d = nc.dram_tensor("wv", [L, 128, H * 128], BF16, kind="ExternalInput")
    wf1_d = nc.dram_tensor("wf1", [L, 128, H * 8 * 128], BF16, kind="ExternalInput")
    wf2_d = nc.dram_tensor("wf2", [L, 128, H * 128], BF16, kind="ExternalInput")
    w1t_d = nc.dram_tensor("w1t", [16, 128, 2 * D], BF16, kind="ExternalInput")
    w2s_d = nc.dram_tensor("w2s", [128, 16], BF16, kind="ExternalInput")
    hb1_d = nc.dram_tensor("hb1", [128, 16], F32, kind="ExternalInput")
    hg_d = nc.dram_tensor("hg", [128, 16], F32, kind="ExternalInput")
    hbt_d = nc.dram_tensor("hbt", [128, 16], F32, kind="ExternalInput")
    gemb_d = nc.dram_tensor("gemb", [128, 8], F32, kind="ExternalInput")
    bemb_d = nc.dram_tensor("bemb", [128, 8], F32, kind="ExternalInput")
    b2_d = nc.dram_tensor("b2", [1, 1], F32, kind="ExternalInput")
    out_d = nc.dram_tensor("out", [1, Bc], F32, kind="ExternalOutput")

    # head staging: k in 0..15 -> (token 69 if k<8 else 70, dtile k%8)
    hstage_d = nc.dram_tensor("hstage", [16, 128, Bc], F32, kind="Internal")

    with tile.TileContext(nc) as tc:
        with tc.tile_pool(name="const", bufs=1) as cpool:
            ones71 = cpool.tile([71, 128], BF16)
            nc.vector.memset(ones71[:], 1.0)
            ones128 = cpool.tile([128, 128], BF16)
            nc.vector.memset(ones128[:], 1.0)
            epsT = cpool.tile([128, 1], F32)
            nc.vector.memset(epsT[:], EPS)
            al02 = cpool.tile([128, 1], F32)
            nc.vector.memset(al02[:], 0.2)

            with (
                tc.tile_pool(name="res", bufs=1) as rpool,
                tc.tile_pool(name="wts", bufs=2) as wpool,
                tc.tile_pool(name="wf1p", bufs=1) as wf1pool,
                tc.tile_pool(name="sb", bufs=2) as sbp,
                tc.tile_pool(name="psA", bufs=2, space="PSUM") as psA,
                tc.tile_pool(name="psB", bufs=2, space="PSUM") as psB,
                tc.tile_pool(name="psC", bufs=2, space="PSUM") as psC,
                tc.tile_pool(name="psD", bufs=1, space="PSUM") as psD,
            ):
                vtab_sb = rpool.tile([KV, D], BF16)
                nc.sync.dma_start(vtab_sb[:], vtab_d[:])
                ident_sb = rpool.tile([128, 128], BF16)
                nc.sync.dma_start(ident_sb[:], ident_d[:])
                cstr_sb = rpool.tile([128, 8 * G * T], F32)
                for k in range(8):
                    nc.sync.dma_start(
                        cstr_sb[:, k * G * T : (k + 1) * G * T], cstr_d[k]
                    )
                if flags["apply_gemb"]:
                    gemb_sb = rpool.tile([128, 8], F32)
                    nc.sync.dma_start(gemb_sb[:], gemb_d[:])
                    bemb_sb = rpool.tile([128, 8], F32)
                    nc.sync.dma_start(bemb_sb[:], bemb_d[:])

                # resident chunk activations
                xb = [
                    rpool.tile([128, CN], BF16, tag=f"xb{k}", name=f"xb{k}")
                    for k in range(8)
                ]

                def embed_tile(c0, g0, N):
                    """Embed + LayerNorm into xf/xb cols [g0*G*T, +N)."""
                    cols = slice(g0 * G * T, g0 * G * T + N)
                    cnt_t = sbp.tile([KV, G * T], BF16, tag="cnt")
                    nc.sync.dma_start(
                        cnt_t[:, :N], cnt_d[:, bass.ds(c0 + g0 * G * T, N)]
                    )
                    sqts = []
                    for k in range(8):
                        e_ps = psA.tile([128, G * T], F32, tag="big", name=f"ep{k}")
                        nc.tensor.matmul(
                            e_ps[:, :N], vtab_sb[:, k * 128 : (k + 1) * 128],
                            cnt_t[:, :N], start=True, stop=True,
                        )
                        nc.vector.tensor_tensor(
                            xb[k][:, cols], e_ps[:, :N],
                            cstr_sb[:, k * G * T : k * G * T + N], ALU.add,
                        )
                        sqt = sbp.tile(
                            [128, G * T], BF16, tag="sq", bufs=3, name=f"sq{k}"
                        )
                        nc.gpsimd.tensor_tensor(
                            sqt[:, :N], xb[k][:, cols], xb[k][:, cols], ALU.mult
                        )
                        sqts.append(sqt)
                    mean_ps = psB.tile([128, G * T], F32, tag="lin")
                    for k in range(8):
                        nc.tensor.matmul(
                            mean_ps[:, :N], ones128[:], xb[k][:, cols],
                            start=(k == 0), stop=(k == 7),
                        )
                    sq_ps = psC.tile([128, G * T], F32, tag="cs")
                    for k in range(8):
                        nc.tensor.matmul(
                            sq_ps[:, :N], ones128[:], sqts[k][:, :N],
                            start=(k == 0), stop=(k == 7),
                        )
                    m1 = sbp.tile([128, G * T], F32, tag="m1", bufs=1)
                    nc.vector.tensor_scalar_mul(m1[:, :N], mean_ps[:, :N], 1.0 / D)
                    msq = sbp.tile([128, G * T], F32, tag="msq", bufs=1)
                    nc.vector.tensor_tensor(msq[:, :N], m1[:, :N], m1[:, :N], ALU.mult)
                    v_t = sbp.tile([128, G * T], F32, tag="v", bufs=1)
                    nc.vector.scalar_tensor_tensor(
                        v_t[:, :N], sq_ps[:, :N], 1.0 / D, msq[:, :N],
                        ALU.mult, ALU.subtract,
                    )
                    # rstd = exp(-0.5*ln(v+eps)) — stays in the exp/ln table set
                    lnv = sbp.tile([128, G * T], F32, tag="lnv", bufs=1)
                    nc.scalar.activation(lnv[:, :N], v_t[:, :N], AF.Ln, bias=epsT[:])
                    rstd = sbp.tile([128, G * T], F32, tag="rstd", bufs=1)
                    nc.scalar.activation(rstd[:, :N], lnv[:, :N], AF.Exp, scale=-0.5)
                    for k in range(8):
                        eng = nc.gpsimd if k % 2 == 0 else nc.vector
                        eng.tensor_tensor(
                            xb[k][:, cols], xb[k][:, cols], m1[:, :N], ALU.subtract
                        )
                        nc.vector.tensor_tensor(
                            xb[k][:, cols], xb[k][:, cols], rstd[:, :N], ALU.mult
                        )
                        if flags["apply_gemb"]:
                            nc.vector.tensor_scalar(
                                xb[k][:, cols], xb[k][:, cols],
                                gemb_sb[:, k : k + 1], bemb_sb[:, k : k + 1],
                                ALU.mult, ALU.add,
                            )

                def layer_tile(wz_sb, wv_sb, wf1_sb, wf2_sb, g0, N, Gn):
                    """Software-pipelined: stage order batches independent
                    heads back-to-back on each in-order engine queue."""
                    t0 = g0 * G * T
                    cols = slice(t0, t0 + N)
                    # ---- attention ----
                    z_sbs = [None] * 8
                    vt_sbs = [None] * 8
                    elns = [None] * 8

                    def z_stage(h):
                        hs = slice(h * 128, (h + 1) * 128)
                        z_ps = psA.tile([128, G * T], F32, tag="big", name=f"zp{h}")
                        nc.tensor.matmul(
                            z_ps[:, :N], wz_sb[:, hs], xb[h][:, cols],
                            start=True, stop=True,
                        )
                        z_sb = sbp.tile(
                            [128, G * T], BF16, tag="z", bufs=3, name=f"zs{h}"
                        )
                        nc.vector.tensor_copy(z_sb[:, :N], z_ps[:, :N])
                        z_sbs[h] = z_sb

                    z_stage(0)
                    z_stage(1)
                    for h in range(8):
                        hs = slice(h * 128, (h + 1) * 128)
                        l_ps = psB.tile([71, G * T], F32, tag="lin", name=f"lp{h}")
                        vt_ps = psD.tile([71, G * 128], F32, tag="vt", name=f"vp{h}")
                        for g in range(Gn):
                            xsg = xb[h][:, t0 + g * T : t0 + (g + 1) * T]
                            gs = slice(g * T, (g + 1) * T)
                            nc.tensor.matmul(
                                l_ps[:, gs], xsg, z_sbs[h][:, gs],
                                start=True, stop=True,
                            )
                            nc.tensor.matmul(
                                vt_ps[:, g * 128 : (g + 1) * 128], xsg, wv_sb[:, hs],
                                start=True, stop=True,
                            )
                        if h + 2 < 8:
                            z_stage(h + 2)
                        el = sbp.tile(
                            [71, G * T], BF16, tag="el", bufs=3, name=f"el{h}"
                        )
                        nc.scalar.activation(el[:, :N], l_ps[:, :N], AF.Exp)
                        vt_sb = sbp.tile(
                            [71, G * 128], BF16, tag="vt", bufs=3, name=f"vs{h}"
                        )
                        nc.vector.tensor_copy(
                            vt_sb[:, : Gn * 128], vt_ps[:, : Gn * 128]
                        )
                        vt_sbs[h] = vt_sb
                        cs_ps = psC.tile([71, G * T], F32, tag="cs", name=f"cp{h}")
                        nc.tensor.matmul(
                            cs_ps[:, :N], ones71[:, :71], el[:, :N],
                            start=True, stop=True,
                        )
                        r_sb = sbp.tile(
                            [71, G * T], F32, tag="r", bufs=3, name=f"rs{h}"
                        )
                        nc.vector.reciprocal_approx_fast(r_sb[:, :N], cs_ps[:, :N])
                        eln = sbp.tile(
                            [71, G * T], BF16, tag="eln", bufs=3, name=f"en{h}"
                        )
                        nc.gpsimd.tensor_tensor(
                            eln[:, :N], el[:, :N], r_sb[:, :N], ALU.mult
                        )
                        elns[h] = eln
                    for h in range(8):
                        y_ps = psA.tile([128, G * T], F32, tag="big", name=f"yp{h}")
                        nc.tensor.matmul(
                            y_ps[:, :N], ident_sb[:], xb[h][:, cols],
                            start=True, stop=False,
                        )
                        for g in range(Gn):
                            gs = slice(g * T, (g + 1) * T)
                            nc.tensor.matmul(
                                y_ps[:, gs], vt_sbs[h][:, g * 128 : (g + 1) * 128],
                                elns[h][:, gs], start=False, stop=(g == Gn - 1),
                            )
                        nc.scalar.activation(xb[h][:, cols], y_ps[:, :N], AF.Copy)
                    # ---- feed-forward (g(m) emitted after f(m+1) mms) ----
                    f_pss = [None] * 8
                    y1s = [None] * 8

                    def f_stage(m):
                        f_ps = psA.tile([128, G * T], F32, tag="big", name=f"fp{m}")
                        for k in range(8):
                            nc.tensor.matmul(
                                f_ps[:, :N],
                                wf1_sb[:, (m * 8 + k) * 128 : (m * 8 + k + 1) * 128],
                                xb[k][:, cols],
                                start=(k == 0), stop=(k == 7),
                            )
                        y1 = sbp.tile(
                            [128, G * T], BF16, tag="y1", bufs=3, name=f"y1{m}"
                        )
                        nc.scalar.activation(
                            y1[:, :N], f_ps[:, :N], AF.Prelu, alpha=al02[:]
                        )
                        f_pss[m] = f_ps
                        y1s[m] = y1

                    def g_stage(m):
                        g_ps = psC.tile([128, G * T], F32, tag="cs", name=f"gp{m}")
                        nc.tensor.matmul(
                            g_ps[:, :N], wf2_sb[:, m * 128 : (m + 1) * 128],
                            y1s[m][:, :N], start=True, stop=True,
                        )
                        y2 = sbp.tile(
                            [128, G * T], BF16, tag="y2", bufs=3, name=f"y2{m}"
                        )
                        nc.scalar.activation(
                            y2[:, :N], g_ps[:, :N], AF.Prelu, alpha=al02[:]
                        )
                        nc.gpsimd.tensor_tensor(
                            xb[m][:, cols], y2[:, :N], xb[m][:, cols], ALU.add
                        )

                    f_stage(0)
                    for m in range(8):
                        if m + 1 < 8:
                            f_stage(m + 1)
                        g_stage(m)

                def head_gather(ocols_start, S):
                    """Copy tokens 69/70 features to hstage[:, :, ocols]."""
                    hg_t = sbp.tile([128, 16 * CHUNK], F32, tag="hg")
                    for k in range(16):
                        tok = 69 if k < 8 else 70
                        src = (
                            xb[k % 8]
                            .rearrange("p (s t) -> p s t", t=T)[:, :S, tok]
                        )
                        nc.vector.tensor_copy(
                            hg_t[:, k * CHUNK : k * CHUNK + S], src
                        )
                    for k in range(16):
                        nc.sync.dma_start(
                            hstage_d[k][:, bass.ds(ocols_start, S)],
                            hg_t[:, k * CHUNK : k * CHUNK + S],
                        )

                def chunk_body(c0, ocols_start, gtiles):
                    """c0: cnt column offset expr; gtiles: list of tile widths."""
                    for g0, Gn in enumerate(gtiles):
                        embed_tile(c0, g0, Gn * T)
                    for l in range(L):
                        wz_sb = wpool.tile([128, H * 128], BF16, tag="wz")
                        nc.sync.dma_start(wz_sb[:], wz_d[l])
                        wv_sb = wpool.tile([128, H * 128], BF16, tag="wv")
                        nc.sync.dma_start(wv_sb[:], wv_d[l])
                        wf2_sb = wpool.tile([128, H * 128], BF16, tag="wf2")
                        nc.sync.dma_start(wf2_sb[:], wf2_d[l])
                        wf1_sb = wf1pool.tile([128, H * 8 * 128], BF16, tag="wf1")
                        nc.sync.dma_start(wf1_sb[:], wf1_d[l])
                        for g0, Gn in enumerate(gtiles):
                            layer_tile(wz_sb, wv_sb, wf1_sb, wf2_sb, g0, Gn * T, Gn)
                    head_gather(ocols_start, sum(gtiles))

                if TIMING_REPEAT > 1:
                    with tc.For_i(0, TIMING_REPEAT):
                        with tc.For_i(0, NCHUNK) as it:
                            chunk_body(it * CN, it * CHUNK, [G, G, G, G])
                else:
                    with tc.For_i(0, NCHUNK) as it:
                        chunk_body(it * CN, it * CHUNK, [G, G, G, G])
                # tail chunk (4 samples, one G=4 tile)
                chunk_body(NCHUNK * CN, NCHUNK * CHUNK, [TAIL])

            # ---------------- head ----------------
            with (
                tc.tile_pool(name="h_sb", bufs=2) as hsb,
                tc.tile_pool(name="h_res", bufs=1) as hres,
                tc.tile_pool(name="h_ps", bufs=2, space="PSUM") as hps,
            ):
                u = hres.tile([128, 16 * Bc], F32)
                for k in range(16):
                    nc.sync.dma_start(u[:, k * Bc : (k + 1) * Bc], hstage_d[k])
                ub = hres.tile([128, 16 * Bc], BF16)
                for k in range(16):
                    ks = slice(k * Bc, (k + 1) * Bc)
                    nc.vector.tensor_copy(ub[:, ks], u[:, ks])
                mean_ps = hps.tile([128, Bc], F32, tag="ln")
                for k in range(16):
                    nc.tensor.matmul(
                        mean_ps[:], ones128[:], ub[:, k * Bc : (k + 1) * Bc],
                        start=(k == 0), stop=(k == 15),
                    )
                sq_ps = hps.tile([128, Bc], F32, tag="ln")
                for k in range(16):
                    sqt = hsb.tile([128, Bc], BF16, tag="sq")
                    ks = slice(k * Bc, (k + 1) * Bc)
                    nc.vector.tensor_tensor(sqt[:], ub[:, ks], ub[:, ks], ALU.mult)
                    nc.tensor.matmul(
                        sq_ps[:], ones128[:], sqt[:], start=(k == 0), stop=(k == 15)
                    )
                m1 = hsb.tile([128, Bc], F32, tag="m1")
                nc.vector.tensor_scalar_mul(m1[:], mean_ps[:], 1.0 / (2 * D))
                msq = hsb.tile([128, Bc], F32, tag="msq")
                nc.vector.tensor_tensor(msq[:], m1[:], m1[:], ALU.mult)
                v_t = hsb.tile([128, Bc], F32, tag="v")
                nc.vector.scalar_tensor_tensor(
                    v_t[:], sq_ps[:], 1.0 / (2 * D), msq[:], ALU.mult, ALU.subtract
                )
                lnv = hsb.tile([128, Bc], F32, tag="lnv")
                nc.scalar.activation(lnv[:], v_t[:], AF.Ln, bias=epsT[:])
                rstd = hsb.tile([128, Bc], F32, tag="rstd")
                nc.scalar.activation(rstd[:], lnv[:], AF.Exp, scale=-0.5)
                if flags["apply_ghead"]:
                    hg_sb = hres.tile([128, 16], F32)
                    nc.sync.dma_start(hg_sb[:], hg_d[:])
                    hbt_sb = hres.tile([128, 16], F32)
                    nc.sync.dma_start(hbt_sb[:], hbt_d[:])
                unb = hres.tile([128, 16 * Bc], BF16)
                for k in range(16):
                    ks = slice(k * Bc, (k + 1) * Bc)
                    xs = hsb.tile([128, Bc], F32, tag="xs")
                    nc.vector.tensor_tensor(xs[:], u[:, ks], m1[:], ALU.subtract)
                    nc.vector.tensor_tensor(xs[:], xs[:], rstd[:], ALU.mult)
                    if flags["apply_ghead"]:
                        nc.vector.tensor_scalar(
                            xs[:], xs[:], hg_sb[:, k : k + 1], hbt_sb[:, k : k + 1],
                            ALU.mult, ALU.add,
                        )
                    nc.vector.tensor_copy(unb[:, ks], xs[:])
                w1_tiles = []
                for k in range(16):
                    wt = hres.tile([128, 2 * D], BF16, tag=f"w1_{k}")
                    nc.sync.dma_start(wt[:], w1t_d[k])
                    w1_tiles.append(wt)
                if flags["use_b1"]:
                    hb1_sb = hres.tile([128, 16], F32)
                    nc.sync.dma_start(hb1_sb[:], hb1_d[:])
                h1 = hres.tile([128, 16 * Bc], BF16)
                for m in range(16):
                    f_ps = hps.tile([128, Bc], F32, tag="f")
                    for k in range(16):
                        nc.tensor.matmul(
                            f_ps[:], w1_tiles[k][:, m * 128 : (m + 1) * 128],
                            unb[:, k * Bc : (k + 1) * Bc],
                            start=(k == 0), stop=(k == 15),
                        )
                    ms = slice(m * Bc, (m + 1) * Bc)
                    bias = hb1_sb[:, m : m + 1] if flags["use_b1"] else 0.0
                    nc.scalar.activation(
                        h1[:, ms], f_ps[:], AF.Prelu, bias=bias, alpha=al02[:]
                    )
                w2_sb = hres.tile([128, 16], BF16)
                nc.sync.dma_start(w2_sb[:], w2s_d[:])
                o_ps = hps.tile([1, Bc], F32, tag="o")
                for k in range(16):
                    nc.tensor.matmul(
                        o_ps[:], w2_sb[:, k : k + 1], h1[:, k * Bc : (k + 1) * Bc],
                        start=(k == 0), stop=(k == 15),
                    )
                # sigmoid(x) = 1/(1+exp(-x)) — avoids a sigmoid table load
                e_sb = hsb.tile([1, Bc], F32, tag="e")
                if flags["use_b2"]:
                    b2_sb = hres.tile([1, 1], F32)
                    nc.sync.dma_start(b2_sb[:], b2_d[:])
                    nb2 = hsb.tile([1, 1], F32, tag="nb2")
                    nc.vector.tensor_scalar_mul(nb2[:], b2_sb[:], -1.0)
                    nc.scalar.activation(
                        e_sb[:], o_ps[:], AF.Exp, scale=-1.0, bias=nb2[:]
                    )
                else:
                    nc.scalar.activation(e_sb[:], o_ps[:], AF.Exp, scale=-1.0)
                d_sb = hsb.tile([1, Bc], F32, tag="d")
                nc.vector.tensor_scalar_add(d_sb[:], e_sb[:], 1.0)
                o_sb = hsb.tile([1, Bc], F32, tag="os")
                nc.vector.reciprocal_approx_fast(o_sb[:], d_sb[:])
                nc.sync.dma_start(out_d[:], o_sb[:])

    return nc


TRACE = False
LAST_RESULT = None


def kernel(**inputs):
    global LAST_RESULT
    in_maps, flags = host_prep(inputs, N_CORES)
    nc = build_program(flags)
    nc.compile()
    res = run_bass_kernel_spmd(
        nc, in_maps, core_ids=list(range(N_CORES)), trace=TRACE
    )
    LAST_RESULT = res
    Bc = flags["Bc"]
    out = np.concatenate([res.results[c]["out"].reshape(Bc, 1) for c in range(N_CORES)])
    return out.astype(np.float32)


# revision 3
# speedup vs baseline: 1.5488x; 1.5488x over previous
"""Trainium2 Bass kernel for the ChessTransformer problem — v2.

Strategy: pure data-parallel over batch (B=2048 -> 256 samples/core on 8
NeuronCores). Per core, samples are processed in SBUF-resident chunks of 28
(4 G-tiles of 7 samples) that flow through embed + all 12 layers + head
feature gather without touching DRAM; only per-layer weights are streamed
(re-loaded once per chunk, ~34 MB/chunk, hidden under compute).

Activations are feature-major: 8 d-tiles of [128, chunk_cols]; the residual
stream lives in bf16 (xb). The attention residual is accumulated in f32 PSUM
by folding an identity matmul into the attention-output accumulation group,
so each layer update rounds to bf16 only once. (layer, tile) steps are
software-pipelined (attention of step i+1 emitted before feed-forward of
step i) so the in-order engine queues always hold independent work.

Attention per head avoids separate q/k projections: host precomputes
A_h = qkv0_h @ qkv1_h^T / sqrt(dh) so scores = x^T A x needs one projection
(z = A^T x) plus per-sample matmuls that share their stationary operand
(x_h,g) between the score matmul and the v matmul. Softmax is over the
partition axis via a ones-matmul column sum; normalization uses the fast
DVE reciprocal and a bf16 multiply.

All ACT usage stays inside the natural_log_exp_and_others table set
(Exp, Ln, Prelu/parametric_relu, Copy, Square): LayerNorm rstd is
exp(-0.5*ln(var+eps)) and the final sigmoid is 1/(1+exp(-x)) via Exp +
DVE reciprocal, so no activation-table reloads occur anywhere.
"""

import sys

sys.path.insert(0, "/opt/trn_rl_repo")

import numpy as np
import ml_dtypes

import concourse.bacc as bacc
import concourse.bass as bass
import concourse.mybir as mybir
from concourse import tile
from concourse.bass_utils import run_bass_kernel_spmd

F32 = mybir.dt.float32
BF16 = mybir.dt.bfloat16
AF = mybir.ActivationFunctionType
ALU = mybir.AluOpType

D = 1024
H = 8
DH = 128
T = 71
KV = 81  # 17 fen rows + 64 pos rows
G = 7  # samples per compute tile (G*T = 497 <= 512 PSUM cols)
N_CORES = 8
EPS = 1e-5

TIMING_REPEAT = 1   # >1: re-run the chunk loop (timing experiments only)
CHUNK = 28          # samples per resident chunk (= 4 G-tiles)
CN = CHUNK * T      # 1988 cols
NCHUNK = 9          # full chunks per core (9*28 = 252)
TAIL = 4            # leftover samples (1 tile of G=4)
BC = NCHUNK * CHUNK + TAIL  # 256


def _bf(a):
    return np.ascontiguousarray(a.astype(ml_dtypes.bfloat16))


def _f32(a):
    return np.ascontiguousarray(a.astype(np.float32))


def host_prep(inputs, n_cores=N_CORES):
    """Build per-core input maps + flags from full-size inputs."""
    fen = np.asarray(inputs["fen"]).astype(np.int64)
    move = np.asarray(inputs["move"]).astype(np.int64)
    B = fen.shape[0]
    Bc = B // n_cores
    L = np.asarray(inputs["qkv"]).shape[0]

    rank_emb = np.asarray(inputs["rank_emb"], np.float32)
    file_emb = np.asarray(inputs["file_emb"], np.float32)
    fen_emb = np.asarray(inputs["fen_emb"], np.float32)
    move_emb = np.asarray(inputs["move_emb"], np.float32)
    abs_emb = np.asarray(inputs["abs_emb"], np.float32)
    qkv = np.asarray(inputs["qkv"], np.float32)
    ff1 = np.asarray(inputs["ff1"], np.float32)
    ff2 = np.asarray(inputs["ff2"], np.float32)
    W1 = np.asarray(inputs["W1"], np.float32)
    b1 = np.asarray(inputs["b1"], np.float32)
    W2 = np.asarray(inputs["W2"], np.float32)
    b2 = np.asarray(inputs["b2"], np.float32)
    lng = np.asarray(inputs["ln_emb_g"], np.float32)
    lnb = np.asarray(inputs["ln_emb_b"], np.float32)
    log = np.asarray(inputs["ln_out_g"], np.float32)
    lob = np.asarray(inputs["ln_out_b"], np.float32)

    pos = (rank_emb + file_emb).reshape(64, D)

    # table + per-token-position constants
    vtab = np.concatenate([fen_emb, 0.58 * pos], axis=0)  # (81, D)
    C = np.empty((T, D), np.float32)
    C[:64] = 0.5 * pos + abs_emb[:64]
    C[64:69] = abs_emb[64:69]
    C[69:71] = 0.58 * move_emb + abs_emb[69:71]

    # count matrix (two-hot embedding weights), cols = b*71 + t
    cnt = np.zeros((KV, B, T), np.float32)
    bidx = np.arange(B)[:, None]
    np.add.at(cnt, (fen[:, :64], bidx, np.arange(64)[None, :]), 0.5)
    np.add.at(cnt, (fen[:, 64:128], bidx, np.arange(64)[None, :]), 0.5)
    np.add.at(cnt, (fen[:, 128:133], bidx, np.arange(64, 69)[None, :]), 1.0)
    np.add.at(cnt, (17 + move, bidx, np.arange(69, 71)[None, :]), 1.0)
    cnt = cnt.reshape(KV, B * T)

    # const replicated G times: feature-major d-tiles [8, 128, G*71]
    Cfm = C.T.reshape(8, 128, T)
    cstr = np.tile(Cfm, (1, 1, G))

    scale = np.sqrt(np.float32(DH))
    # fused score weight: z = wz.T @ x gives scores = x^T z = q.k/scale
    # wz[l,:,h] = qkv0 @ qkv1^T / scale  (as lhsT: out = lhsT.T @ rhs)
    wz = np.einsum("lhic,lhjc->lihj", qkv[:, 0], qkv[:, 1]).reshape(L, 128, H * 128)
    wz = wz / scale
    # v weight (moving operand): vt[s,d'] = sum_c x[c,s] qkv2[c,d']
    wv = qkv[:, 2].transpose(0, 2, 1, 3).reshape(L, 128, H * 128)
    wf1 = (
        ff1.reshape(L, H, 8, 128, DH).transpose(0, 3, 1, 2, 4).reshape(L, 128, H * 8 * 128)
    )
    wf2 = ff2.transpose(0, 2, 1, 3).reshape(L, 128, H * 128)

    w1t = W1.T.reshape(16, 128, 2 * D)  # [k, p, out]
    w2s = W2.reshape(16, 128).T  # [128, 16]
    hb1 = b1.reshape(16, 128).T  # [128, 16]
    hg = log.reshape(16, 128).T
    hbt = lob.reshape(16, 128).T
    gemb = lng.reshape(8, 128).T  # [128, 8]
    bemb = lnb.reshape(8, 128).T

    flags = dict(
        apply_gemb=not (np.all(lng == 1.0) and np.all(lnb == 0.0)),
        apply_ghead=not (np.all(log == 1.0) and np.all(lob == 0.0)),
        use_b1=bool(np.any(b1 != 0.0)),
        use_b2=bool(np.any(b2 != 0.0)),
        Bc=Bc,
        L=L,
    )

    shared = {
        "ident": _bf(np.eye(128, dtype=np.float32)),
        "vtab": _bf(vtab),
        "cstr": _f32(cstr),
        "wz": _bf(wz),
        "wv": _bf(wv),
        "wf1": _bf(wf1),
        "wf2": _bf(wf2),
        "w1t": _bf(w1t),
        "w2s": _bf(w2s),
        "hb1": _f32(hb1),
        "hg": _f32(hg),
        "hbt": _f32(hbt),
        "gemb": _f32(gemb),
        "bemb": _f32(bemb),
        "b2": _f32(b2.reshape(1, 1)),
    }
    cnt_bf = _bf(cnt)
    in_maps = []
    for c in range(n_cores):
        m = dict(shared)
        m["cnt"] = np.ascontiguousarray(cnt_bf[:, c * Bc * T : (c + 1) * Bc * T])
        in_maps.append(m)
    return in_maps, flags


def build_program(flags):
    """Emit the full per-core program."""
    Bc = flags["Bc"]
    L = flags["L"]
    TOK = Bc * T
    assert Bc == BC

    nc = bacc.Bacc("TRN2", target_bir_lowering=False, debug=False)

    cnt_d = nc.dram_tensor("cnt", [KV, TOK], BF16, kind="ExternalInput")
    ident_d = nc.dram_tensor("ident", [128, 128], BF16, kind="ExternalInput")
    vtab_d = nc.dram_tensor("vtab", [KV, D], BF16, kind="ExternalInput")
    cstr_d = nc.dram_tensor("cstr", [8, 128, G * T], F32, kind="ExternalInput")
    wz_d = nc.dram_tensor("wz", [L, 128, H * 128], BF16, kind="ExternalInput")
    wv_d = nc.dram_tensor("wv", [L, 128, H * 128], BF16, kind="ExternalInput")
    wf1_d = nc.dram_tensor("wf1", [L, 128, H * 8 * 128], BF16, kind="ExternalInput")
    wf2_d = nc.dram_tensor("wf2", [L, 128, H * 128], BF16, kind="ExternalInput")
    w1t_d = nc.dram_tensor("w1t", [16, 128, 2 * D], BF16, kind="ExternalInput")
    w2s_d = nc.dram_tensor("w2s", [128, 16], BF16, kind="ExternalInput")
    hb1_d = nc.dram_tensor("hb1", [128, 16], F32, kind="ExternalInput")
    hg_d = nc.dram_tensor("hg", [128, 16], F32, kind="ExternalInput")
    hbt_d = nc.dram_tensor("hbt", [128, 16], F32, kind="ExternalInput")
    gemb_d = nc.dram_tensor("gemb", [128, 8], F32, kind="ExternalInput")
    bemb_d = nc.dram_tensor("bemb", [128, 8], F32, kind="ExternalInput")
    b2_d = nc.dram_tensor("b2", [1, 1], F32, kind="ExternalInput")
    out_d = nc.dram_tensor("out", [1, Bc], F32, kind="ExternalOutput")

    # head staging: k in 0..15 -> (token 69 if k<8 else 70, dtile k%8)
    hstage_d = nc.dram_tensor("hstage", [16, 128, Bc], F32, kind="Internal")

    with tile.TileContext(nc) as tc:
        with tc.tile_pool(name="const", bufs=1) as cpool:
            ones71 = cpool.tile([71, 128], BF16)
            nc.vector.memset(ones71[:], 1.0)
            ones128 = cpool.tile([128, 128], BF16)
            nc.vector.memset(ones128[:], 1.0)
            epsT = cpool.tile([128, 1], F32)
            nc.vector.memset(epsT[:], EPS)
            al02 = cpool.tile([128, 1], F32)
            nc.vector.memset(al02[:], 0.2)

            with (
                tc.tile_pool(name="res", bufs=1) as rpool,
                tc.tile_pool(name="wts", bufs=2) as wpool,
                tc.tile_pool(name="wf1p", bufs=2) as wf1pool,
                tc.tile_pool(name="sb", bufs=2) as sbp,
                tc.tile_pool(name="psA", bufs=3, space="PSUM") as psA,
                tc.tile_pool(name="psB", bufs=1, space="PSUM") as psB,
                tc.tile_pool(name="psC", bufs=2, space="PSUM") as psC,
                tc.tile_pool(name="psD", bufs=1, space="PSUM") as psD,
            ):
                vtab_sb = rpool.tile([KV, D], BF16)
                nc.sync.dma_start(vtab_sb[:], vtab_d[:])
                ident_sb = rpool.tile([128, 128], BF16)
                nc.sync.dma_start(ident_sb[:], ident_d[:])
                cstr_sb = rpool.tile([128, 8 * G * T], F32)
                for k in range(8):
                    nc.sync.dma_start(
                        cstr_sb[:, k * G * T : (k + 1) * G * T], cstr_d[k]
                    )
                if flags["apply_gemb"]:
                    gemb_sb = rpool.tile([128, 8], F32)
                    nc.sync.dma_start(gemb_sb[:], gemb_d[:])
                    bemb_sb = rpool.tile([128, 8], F32)
                    nc.sync.dma_start(bemb_sb[:], bemb_d[:])

                # resident chunk activations
                xb = [
                    rpool.tile([128, CN], BF16, tag=f"xb{k}", name=f"xb{k}")
                    for k in range(8)
                ]

                def embed_tile(c0, g0, N):
                    """Embed + LayerNorm into xf/xb cols [g0*G*T, +N)."""
                    cols = slice(g0 * G * T, g0 * G * T + N)
                    cnt_t = sbp.tile([KV, G * T], BF16, tag="cnt")
                    nc.sync.dma_start(
                        cnt_t[:, :N], cnt_d[:, bass.ds(c0 + g0 * G * T, N)]
                    )
                    sqts = []
                    for k in range(8):
                        e_ps = psA.tile([128, G * T], F32, tag="big", name=f"ep{k}")
                        nc.tensor.matmul(
                            e_ps[:, :N], vtab_sb[:, k * 128 : (k + 1) * 128],
                            cnt_t[:, :N], start=True, stop=True,
                        )
                        nc.vector.tensor_tensor(
                            xb[k][:, cols], e_ps[:, :N],
                            cstr_sb[:, k * G * T : k * G * T + N], ALU.add,
                        )
                        sqt = sbp.tile(
                            [128, G * T], BF16, tag="sq", bufs=3, name=f"sq{k}"
                        )
                        nc.gpsimd.tensor_tensor(
                            sqt[:, :N], xb[k][:, cols], xb[k][:, cols], ALU.mult
                        )
                        sqts.append(sqt)
                    mean_ps = psB.tile([128, G * T], F32, tag="lin")
                    for k in range(8):
                        nc.tensor.matmul(
                            mean_ps[:, :N], ones128[:], xb[k][:, cols],
                            start=(k == 0), stop=(k == 7),
                        )
                    sq_ps = psC.tile([128, G * T], F32, tag="cs")
                    for k in range(8):
                        nc.tensor.matmul(
                            sq_ps[:, :N], ones128[:], sqts[k][:, :N],
                            start=(k == 0), stop=(k == 7),
                        )
                    m1 = sbp.tile([128, G * T], F32, tag="m1", bufs=1)
                    nc.vector.tensor_scalar_mul(m1[:, :N], mean_ps[:, :N], 1.0 / D)
                    msq = sbp.tile([128, G * T], F32, tag="msq", bufs=1)
                    nc.vector.tensor_tensor(msq[:, :N], m1[:, :N], m1[:, :N], ALU.mult)
                    v_t = sbp.tile([128, G * T], F32, tag="v", bufs=1)
                    nc.vector.scalar_tensor_tensor(
                        v_t[:, :N], sq_ps[:, :N], 1.0 / D, msq[:, :N],
                        ALU.mult, ALU.subtract,
                    )
                    # rstd = exp(-0.5*ln(v+eps)) — stays in the exp/ln table set
                    lnv = sbp.tile([128, G * T], F32, tag="lnv", bufs=1)
                    nc.scalar.activation(lnv[:, :N], v_t[:, :N], AF.Ln, bias=epsT[:])
                    rstd = sbp.tile([128, G * T], F32, tag="rstd", bufs=1)
                    nc.scalar.activation(rstd[:, :N], lnv[:, :N], AF.Exp, scale=-0.5)
                    for k in range(8):
                        eng = nc.gpsimd if k % 2 == 0 else nc.vector
                        eng.tensor_tensor(
                            xb[k][:, cols], xb[k][:, cols], m1[:, :N], ALU.subtract
                        )
                        nc.vector.tensor_tensor(
                            xb[k][:, cols], xb[k][:, cols], rstd[:, :N], ALU.mult
                        )
                        if flags["apply_gemb"]:
                            nc.vector.tensor_scalar(
                                xb[k][:, cols], xb[k][:, cols],
                                gemb_sb[:, k : k + 1], bemb_sb[:, k : k + 1],
                                ALU.mult, ALU.add,
                            )

                def attn_tile(wz_sb, wv_sb, g0, N, Gn):
                    """Software-pipelined: stage order batches independent
                    heads back-to-back on each in-order engine queue."""
                    t0 = g0 * G * T
                    cols = slice(t0, t0 + N)
                    # ---- attention ----
                    z_sbs = [None] * 8
                    vt_sbs = [None] * 8
                    elns = [None] * 8

                    def z_stage(h):
                        hs = slice(h * 128, (h + 1) * 128)
                        z_ps = psA.tile([128, G * T], F32, tag="big", name=f"zp{h}")
                        nc.tensor.matmul(
                            z_ps[:, :N], wz_sb[:, hs], xb[h][:, cols],
                            start=True, stop=True,
                        )
                        z_sb = sbp.tile(
                            [128, G * T], BF16, tag="z", bufs=3, name=f"zs{h}"
                        )
                        nc.vector.tensor_copy(z_sb[:, :N], z_ps[:, :N])
                        z_sbs[h] = z_sb

                    z_stage(0)
                    z_stage(1)
                    for h in range(8):
                        hs = slice(h * 128, (h + 1) * 128)
                        l_ps = psB.tile([71, G * T], F32, tag="lin", name=f"lp{h}")
                        vt_ps = psD.tile([71, G * 128], F32, tag="vt", name=f"vp{h}")
                        for g in range(Gn):
                            xsg = xb[h][:, t0 + g * T : t0 + (g + 1) * T]
                            gs = slice(g * T, (g + 1) * T)
                            nc.tensor.matmul(
                                l_ps[:, gs], xsg, z_sbs[h][:, gs],
                                start=True, stop=True,
                            )
                            nc.tensor.matmul(
                                vt_ps[:, g * 128 : (g + 1) * 128], xsg, wv_sb[:, hs],
                                start=True, stop=True,
                            )
                        if h + 2 < 8:
                            z_stage(h + 2)
                        el = sbp.tile(
                            [71, G * T], BF16, tag="el", bufs=3, name=f"el{h}"
                        )
                        nc.scalar.activation(el[:, :N], l_ps[:, :N], AF.Exp)
                        vt_sb = sbp.tile(
                            [71, G * 128], BF16, tag="vt", bufs=3, name=f"vs{h}"
                        )
                        nc.vector.tensor_copy(
                            vt_sb[:, : Gn * 128], vt_ps[:, : Gn * 128]
                        )
                        vt_sbs[h] = vt_sb
                        cs_ps = psC.tile([71, G * T], F32, tag="cs", name=f"cp{h}")
                        nc.tensor.matmul(
                            cs_ps[:, :N], ones71[:, :71], el[:, :N],
                            start=True, stop=True,
                        )
                        r_sb = sbp.tile(
                            [71, G * T], F32, tag="r", bufs=3, name=f"rs{h}"
                        )
                        nc.vector.reciprocal_approx_fast(r_sb[:, :N], cs_ps[:, :N])
                        eln = sbp.tile(
                            [71, G * T], BF16, tag="eln", bufs=3, name=f"en{h}"
                        )
                        nc.gpsimd.tensor_tensor(
                            eln[:, :N], el[:, :N], r_sb[:, :N], ALU.mult
                        )
                        elns[h] = eln
                    for h in range(8):
                        y_ps = psA.tile([128, G * T], F32, tag="big", name=f"yp{h}")
                        nc.tensor.matmul(
                            y_ps[:, :N], ident_sb[:], xb[h][:, cols],
                            start=True, stop=False,
                        )
                        for g in range(Gn):
                            gs = slice(g * T, (g + 1) * T)
                            nc.tensor.matmul(
                                y_ps[:, gs], vt_sbs[h][:, g * 128 : (g + 1) * 128],
                                elns[h][:, gs], start=False, stop=(g == Gn - 1),
                            )
                        nc.scalar.activation(xb[h][:, cols], y_ps[:, :N], AF.Copy)

                def ff_tile(wf1_sb, wf2_sb, g0, N, Gn):
                    """Feed-forward for one tile (g(m) emitted after f(m+1))."""
                    t0 = g0 * G * T
                    cols = slice(t0, t0 + N)
                    f_pss = [None] * 8
                    y1s = [None] * 8

                    def f_stage(m):
                        f_ps = psA.tile([128, G * T], F32, tag="big", name=f"fp{m}")
                        for k in range(8):
                            nc.tensor.matmul(
                                f_ps[:, :N],
                                wf1_sb[:, (m * 8 + k) * 128 : (m * 8 + k + 1) * 128],
                                xb[k][:, cols],
                                start=(k == 0), stop=(k == 7),
                            )
                        y1 = sbp.tile(
                            [128, G * T], BF16, tag="y1", bufs=3, name=f"y1{m}"
                        )
                        nc.scalar.activation(
                            y1[:, :N], f_ps[:, :N], AF.Prelu, alpha=al02[:]
                        )
                        f_pss[m] = f_ps
                        y1s[m] = y1

                    def g_stage(m):
                        g_ps = psC.tile([128, G * T], F32, tag="cs", name=f"gp{m}")
                        nc.tensor.matmul(
                            g_ps[:, :N], wf2_sb[:, m * 128 : (m + 1) * 128],
                            y1s[m][:, :N], start=True, stop=True,
                        )
                        y2 = sbp.tile(
                            [128, G * T], BF16, tag="y2", bufs=3, name=f"y2{m}"
                        )
                        nc.scalar.activation(
                            y2[:, :N], g_ps[:, :N], AF.Prelu, alpha=al02[:]
                        )
                        nc.gpsimd.tensor_tensor(
                            xb[m][:, cols], y2[:, :N], xb[m][:, cols], ALU.add
                        )

                    f_stage(0)
                    for m in range(8):
                        if m + 1 < 8:
                            f_stage(m + 1)
                        g_stage(m)

                def head_gather(ocols_start, S):
                    """Copy tokens 69/70 features to hstage[:, :, ocols]."""
                    hg_t = sbp.tile([128, 16 * CHUNK], F32, tag="hg")
                    for k in range(16):
                        tok = 69 if k < 8 else 70
                        src = (
                            xb[k % 8]
                            .rearrange("p (s t) -> p s t", t=T)[:, :S, tok]
                        )
                        nc.vector.tensor_copy(
                            hg_t[:, k * CHUNK : k * CHUNK + S], src
                        )
                    for k in range(16):
                        nc.sync.dma_start(
                            hstage_d[k][:, bass.ds(ocols_start, S)],
                            hg_t[:, k * CHUNK : k * CHUNK + S],
                        )

                def chunk_body(c0, ocols_start, gtiles):
                    """c0: cnt column offset expr; gtiles: list of tile widths.

                    (layer, tile) steps are software-pipelined: attention of
                    step i+1 is emitted before feed-forward of step i, so the
                    in-order PE queue always has independent work.
                    """
                    for g0, Gn in enumerate(gtiles):
                        embed_tile(c0, g0, Gn * T)
                    stream = [(l, g0) for l in range(L) for g0 in range(len(gtiles))]
                    wts = {}

                    def load_w(l):
                        wz_sb = wpool.tile([128, H * 128], BF16, tag="wz")
                        nc.sync.dma_start(wz_sb[:], wz_d[l])
                        wv_sb = wpool.tile([128, H * 128], BF16, tag="wv")
                        nc.sync.dma_start(wv_sb[:], wv_d[l])
                        wf2_sb = wpool.tile([128, H * 128], BF16, tag="wf2")
                        nc.sync.dma_start(wf2_sb[:], wf2_d[l])
                        wf1_sb = wf1pool.tile([128, H * 8 * 128], BF16, tag="wf1")
                        nc.sync.dma_start(wf1_sb[:], wf1_d[l])
                        wts[l] = (wz_sb, wv_sb, wf1_sb, wf2_sb)

                    def do_attn(i):
                        l, g0 = stream[i]
                        if g0 == 0:
                            load_w(l)
                        wz_sb, wv_sb, _, _ = wts[l]
                        Gn = gtiles[g0]
                        attn_tile(wz_sb, wv_sb, g0, Gn * T, Gn)

                    do_attn(0)
                    for i in range(len(stream)):
                        if i + 1 < len(stream):
                            do_attn(i + 1)
                        l, g0 = stream[i]
                        _, _, wf1_sb, wf2_sb = wts[l]
                        Gn = gtiles[g0]
                        ff_tile(wf1_sb, wf2_sb, g0, Gn * T, Gn)
                    head_gather(ocols_start, sum(gtiles))

                if TIMING_REPEAT > 1:
                    with tc.For_i(0, TIMING_REPEAT):
                        with tc.For_i(0, NCHUNK) as it:
                            chunk_body(it * CN, it * CHUNK, [G, G, G, G])
                else:
                    with tc.For_i(0, NCHUNK) as it:
                        chunk_body(it * CN, it * CHUNK, [G, G, G, G])
                # tail chunk (4 samples, one G=4 tile)
                chunk_body(NCHUNK * CN, NCHUNK * CHUNK, [TAIL])

            # ---------------- head ----------------
            with (
                tc.tile_pool(name="h_sb", bufs=2) as hsb,
                tc.tile_pool(name="h_res", bufs=1) as hres,
                tc.tile_pool(name="h_ps", bufs=2, space="PSUM") as hps,
            ):
                u = hres.tile([128, 16 * Bc], F32)
                for k in range(16):
                    nc.sync.dma_start(u[:, k * Bc : (k + 1) * Bc], hstage_d[k])
                ub = hres.tile([128, 16 * Bc], BF16)
                for k in range(16):
                    ks = slice(k * Bc, (k + 1) * Bc)
                    nc.vector.tensor_copy(ub[:, ks], u[:, ks])
                mean_ps = hps.tile([128, Bc], F32, tag="ln")
                for k in range(16):
                    nc.tensor.matmul(
                        mean_ps[:], ones128[:], ub[:, k * Bc : (k + 1) * Bc],
                        start=(k == 0), stop=(k == 15),
                    )
                sq_ps = hps.tile([128, Bc], F32, tag="ln")
                for k in range(16):
                    sqt = hsb.tile([128, Bc], BF16, tag="sq")
                    ks = slice(k * Bc, (k + 1) * Bc)
                    nc.vector.tensor_tensor(sqt[:], ub[:, ks], ub[:, ks], ALU.mult)
                    nc.tensor.matmul(
                        sq_ps[:], ones128[:], sqt[:], start=(k == 0), stop=(k == 15)
                    )
                m1 = hsb.tile([128, Bc], F32, tag="m1")
                nc.vector.tensor_scalar_mul(m1[:], mean_ps[:], 1.0 / (2 * D))
                msq = hsb.tile([128, Bc], F32, tag="msq")
                nc.vector.tensor_tensor(msq[:], m1[:], m1[:], ALU.mult)
                v_t = hsb.tile([128, Bc], F32, tag="v")
                nc.vector.scalar_tensor_tensor(
                    v_t[:], sq_ps[:], 1.0 / (2 * D), msq[:], ALU.mult, ALU.subtract
                )
                lnv = hsb.tile([128, Bc], F32, tag="lnv")
                nc.scalar.activation(lnv[:], v_t[:], AF.Ln, bias=epsT[:])
                rstd = hsb.tile([128, Bc], F32, tag="rstd")
                nc.scalar.activation(rstd[:], lnv[:], AF.Exp, scale=-0.5)
                if flags["apply_ghead"]:
                    hg_sb = hres.tile([128, 16], F32)
                    nc.sync.dma_start(hg_sb[:], hg_d[:])
                    hbt_sb = hres.tile([128, 16], F32)
                    nc.sync.dma_start(hbt_sb[:], hbt_d[:])
                unb = hres.tile([128, 16 * Bc], BF16)
                for k in range(16):
                    ks = slice(k * Bc, (k + 1) * Bc)
                    xs = hsb.tile([128, Bc], F32, tag="xs")
                    nc.vector.tensor_tensor(xs[:], u[:, ks], m1[:], ALU.subtract)
                    nc.vector.tensor_tensor(xs[:], xs[:], rstd[:], ALU.mult)
                    if flags["apply_ghead"]:
                        nc.vector.tensor_scalar(
                            xs[:], xs[:], hg_sb[:, k : k + 1], hbt_sb[:, k : k + 1],
                            ALU.mult, ALU.add,
                        )
                    nc.vector.tensor_copy(unb[:, ks], xs[:])
                w1_tiles = []
                for k in range(16):
                    wt = hres.tile([128, 2 * D], BF16, tag=f"w1_{k}")
                    nc.sync.dma_start(wt[:], w1t_d[k])
                    w1_tiles.append(wt)
                if flags["use_b1"]:
                    hb1_sb = hres.tile([128, 16], F32)
                    nc.sync.dma_start(hb1_sb[:], hb1_d[:])
                h1 = hres.tile([128, 16 * Bc], BF16)
                for m in range(16):
                    f_ps = hps.tile([128, Bc], F32, tag="f")
                    for k in range(16):
                        nc.tensor.matmul(
                            f_ps[:], w1_tiles[k][:, m * 128 : (m + 1) * 128],
                            unb[:, k * Bc : (k + 1) * Bc],
                            start=(k == 0), stop=(k == 15),
                        )
                    ms = slice(m * Bc, (m + 1) * Bc)
                    bias = hb1_sb[:, m : m + 1] if flags["use_b1"] else 0.0
                    nc.scalar.activation(
                        h1[:, ms], f_ps[:], AF.Prelu, bias=bias, alpha=al02[:]
                    )
                w2_sb = hres.tile([128, 16], BF16)
                nc.sync.dma_start(w2_sb[:], w2s_d[:])
                o_ps = hps.tile([1, Bc], F32, tag="o")
                for k in range(16):
                    nc.tensor.matmul(
                        o_ps[:], w2_sb[:, k : k + 1], h1[:, k * Bc : (k + 1) * Bc],
                        start=(k == 0), stop=(k == 15),
                    )
                # sigmoid(x) = 1/(1+exp(-x)) — avoids a sigmoid table load
                e_sb = hsb.tile([1, Bc], F32, tag="e")
                if flags["use_b2"]:
                    b2_sb = hres.tile([1, 1], F32)
                    nc.sync.dma_start(b2_sb[:], b2_d[:])
                    nb2 = hsb.tile([1, 1], F32, tag="nb2")
                    nc.vector.tensor_scalar_mul(nb2[:], b2_sb[:], -1.0)
                    nc.scalar.activation(
                        e_sb[:], o_ps[:], AF.Exp, scale=-1.0, bias=nb2[:]
                    )
                else:
                    nc.scalar.activation(e_sb[:], o_ps[:], AF.Exp, scale=-1.0)
                d_sb = hsb.tile([1, Bc], F32, tag="d")
                nc.vector.tensor_scalar_add(d_sb[:], e_sb[:], 1.0)
                o_sb = hsb.tile([1, Bc], F32, tag="os")
                nc.vector.reciprocal_approx_fast(o_sb[:], d_sb[:])
                nc.sync.dma_start(out_d[:], o_sb[:])

    return nc


TRACE = False
LAST_RESULT = None


def kernel(**inputs):
    global LAST_RESULT
    in_maps, flags = host_prep(inputs, N_CORES)
    nc = build_program(flags)
    nc.compile()
    res = run_bass_kernel_spmd(
        nc, in_maps, core_ids=list(range(N_CORES)), trace=TRACE
    )
    LAST_RESULT = res
    Bc = flags["Bc"]
    out = np.concatenate([res.results[c]["out"].reshape(Bc, 1) for c in range(N_CORES)])
    return out.astype(np.float32)
